# revision 41
# baseline (speedup 1.0000x reference)
"""BitConv2d inference kernel for Trainium2 (8 NeuronCores, SPMD).

Problem: y = conv2d(x, w_q.float(), stride=1, pad=1) * s + bias
  x:    (32, 128, 56, 56) f32
  w_q:  (256, 128, 3, 3) ternary {-1,0,+1} (int8 or int32)
  s:    (256, 1, 1) f32
  bias: (256,) f32
  y:    (32, 256, 56, 56) f32

Strategy: data-parallel over batch (4 images per core) + Winograd F(2,3)
along H. The 3 kh-taps collapse into 4 host-precomputed fp16 "streams"
per image (S1..S4 = +/- combinations of adjacent padded rows), so each
pair of output rows needs only 4 matmul products instead of 6:

  bank1 = M1 = sum_kw V1[kw]^T S1(shift kw)      (3 matmuls)
  bank2 = M2                                      (3)
  bank3 = -M3  (V3 pre-negated on host)           (3)
  bank4 = M4                                      (3)
  even rows E = M1 + M2 + M3 + b,  odd rows O = M2 - M3 - M4 + b

12 matmuls per 16 output rows vs 18 direct = 1.5x fewer PE cycles.
The kw taps reuse the baseline's stride-57 shifted-slice trick (stream
rows are 57 wide; the zero column between rows is the shared pad).

Combine per group of 7 row-pairs (free dim 399 = 7x57, one PSUM bank):
  ACT: C2 = Id(bank2 + bias) -> fp16   (bias rides into both parities)
       B  = Id([bank3; bank4])  -> fp16, rows interleaved
  DVE: T(even) = bank1 + C2, T(odd) = bank3 + C2   (rows interleaved)
       stage   = T - B        (dense fp16 2x op; E = T-(-M3), O = T-M4)
All junk pad columns are dropped at the PSUM->SBUF boundary, so the
staging tile and the output DMA are fully contiguous (14 rows x 224B
per partition per group), line-rate both sides.

Scales s are folded into the transformed weights on the host.
Per-core per-group engine budget @2.4GHz: PE 1995ns, DVE ~1700ns,
ACT ~1570ns -> PE-bound, ~32 groups ~= 67-70us (vs 114.5us baseline).
"""

import os
from contextlib import ExitStack

import numpy as np

import concourse.bass as bass
import concourse.mybir as mybir
from concourse import bacc

# Problem constants (hardcoded per contract)
N_IMG, C_IN, C_OUT, H, W = 32, 128, 256, 56, 56
N_CORES = 8
IMG = N_IMG // N_CORES  # 4 images per core
IMG_PER_CORE = IMG  # test.py compat
PASSES = 1  # test.py compat (no residual pass needed: rel err ~1e-3)
N_CH = C_OUT // 128  # 2 cout chunks

SROW = 57  # stream row stride (56 cols + shared pad col)
NPAIR = 28  # output row pairs per image
GQ = 7  # row pairs per group
NGRP = NPAIR // GQ  # 4 groups per (img, chunk)
SFREE = GQ * SROW  # 399, matmul free dim (<= 512 fp32 PSUM bank)
SCOLS = NPAIR * SROW + 8  # 1604: stream cols + tap-overshoot slack
STG = 2 * GQ * W  # 784: staging elems per group (14 rows x 56)
IMG_RUN = int(os.environ.get("WINO_IMGS", "4"))  # debug: images to process
N_GRP = int(os.environ.get("WINO_GRPS", str(IMG_RUN * N_CH * NGRP)))
N_WU = int(os.environ.get("WINO_WARMUP", "30"))
KO = set(os.environ.get("WINO_KO", "").split(","))  # debug knockouts
N_WARMUP = 12
WU_FREE = 256

f16 = mybir.dt.float16
f32 = mybir.dt.float32


def build_nc() -> bacc.Bacc:
    nc = bacc.Bacc("TRN2", target_bir_lowering=False, debug=False)

    xs = nc.dram_tensor("xs", [IMG, 4, C_IN, SCOLS], f16,
                        kind="ExternalInput").ap()
    wt = nc.dram_tensor("wt", [C_IN, 24 * 128], f16, kind="ExternalInput").ap()
    bv = nc.dram_tensor("bv", [128, N_CH], f32, kind="ExternalInput").ap()
    y = nc.dram_tensor("y", [IMG, C_OUT, H, W], f32, kind="ExternalOutput").ap()

    # static SBUF
    xs_t = [nc.alloc_sbuf_tensor(f"sxs{i}", [C_IN, 4 * SCOLS], f16).ap()
            for i in range(IMG)]
    wt_t = nc.alloc_sbuf_tensor("swt", [C_IN, 24 * 128], f16).ap()
    bv_t = nc.alloc_sbuf_tensor("sbv", [128, N_CH], f32).ap()
    wu = nc.alloc_sbuf_tensor("swu", [128, WU_FREE], f16).ap()
    scr = nc.alloc_sbuf_tensor("sscr", [128, 16], f16).ap()
    scr34 = nc.alloc_sbuf_tensor("sscr34", [128, STG], f32).ap()
    c2_t = [nc.alloc_sbuf_tensor(f"sc2{h}", [128, GQ * W], f16).ap()
            for h in range(2)]
    t_t = [nc.alloc_sbuf_tensor(f"st{h}", [128, STG], f16).ap()
           for h in range(2)]
    b_t = [nc.alloc_sbuf_tensor(f"sb{h}", [128, STG], f16).ap()
           for h in range(2)]
    st_t = [nc.alloc_sbuf_tensor(f"sst{j}", [128, NGRP * STG], f32).ap()
            for j in range(4)]
    ps = nc.alloc_psum_tensor("ps", [128, 4096], f32).ap()

    def bank_mm(h, m):  # matmul target: one PSUM bank, dense
        base = (4 * h + m) * 512
        return ps[:, base:base + SFREE]

    def bank_rd(h, m):  # junk-column-free read view [p, 7, 56]
        base = (4 * h + m) * 512
        return ps[:, base:base + SFREE].rearrange(
            "p (r c) -> p r c", c=SROW)[:, :, 0:W]

    def b34_in(h):  # banks 3,4 as [p, 2, 7, 56]
        base = (4 * h + 2) * 512
        return ps[:, base:base + 1024].rearrange(
            "p (b x) -> p b x", b=2)[:, :, 0:SFREE].rearrange(
            "p b (r c) -> p b r c", c=SROW)[:, :, :, 0:W]

    def il_view(tile):  # [128, 784] -> row-interleaved [p, parity, 7, 56]
        return tile.rearrange("p (r t c) -> p t r c", t=2, c=W)

    def c2_view(h):
        return c2_t[h].rearrange("p (r c) -> p r c", c=W)

    wu_ps = ps[:, 7 * 512:7 * 512 + WU_FREE]

    Id = mybir.ActivationFunctionType.Identity

    # per-image group sequence: (chunk, quad) ordered so late-arriving
    # input chunks (b2) and c1 weights are needed as late as possible
    SEQ = [(0, 0), (0, 1), (1, 0), (1, 1), (0, 2), (0, 3), (1, 2), (1, 3)]

    def gidx(g):
        c, q = SEQ[g % 8]
        return g // 8, c, q, g % 2  # img, chunk, quad, psum half

    with ExitStack() as ctx:
        s_wu = ctx.enter_context(nc.semaphore("s_wu"))
        s_wt = [ctx.enter_context(nc.semaphore(f"s_wt{k}")) for k in range(6)]
        s_x0p = [ctx.enter_context(nc.semaphore(f"s_x0p{m}")) for m in range(4)]
        s_x0b1 = ctx.enter_context(nc.semaphore("s_x0b1"))
        s_x0b2 = ctx.enter_context(nc.semaphore("s_x0b2"))
        s_x = [None] + [ctx.enter_context(nc.semaphore(f"s_x{i}"))
                        for i in range(1, IMG)]
        s_cst = ctx.enter_context(nc.semaphore("s_cst"))
        s_mm = ctx.enter_context(nc.semaphore("s_mm"))
        s_c2 = ctx.enter_context(nc.semaphore("s_c2"))
        s_b = ctx.enter_context(nc.semaphore("s_b"))
        s_d2 = ctx.enter_context(nc.semaphore("s_d2"))
        s_d34 = ctx.enter_context(nc.semaphore("s_d34"))
        s_out = [ctx.enter_context(nc.semaphore(f"s_out{j}"))
                 for j in range(4)]
        block = ctx.enter_context(nc.Block())

        @block.scalar
        def _(eng):
            # input DMAs on the ACT HWDGE ring; critical prefix first.
            # weight blocks ordered (chunk, m, kw); group 0 consumes
            # m-banks in order, so stage the DMAs to match.
            # ring A (ACT HWDGE): img0 streams 1-2, all weights, img2
            eng.dma_start(out=wt_t[:, 0:384], in_=wt[:, 0:384]).then_inc(s_wt[0], 16)
            for m in range(2):
                eng.dma_start(out=xs_t[0][:, m * SCOLS:m * SCOLS + 456],
                              in_=xs[0, m][:, 0:456]).then_inc(s_x0p[m], 16)
            eng.dma_start(out=wt_t[:, 384:1536], in_=wt[:, 384:1536]).then_inc(s_wt[1], 16)
            for m in range(2):
                eng.dma_start(out=xs_t[0][:, m * SCOLS + 456:m * SCOLS + 860],
                              in_=xs[0, m][:, 456:860]).then_inc(s_x0b1, 16)
            for m in range(2):
                eng.dma_start(out=xs_t[0][:, m * SCOLS + 860:(m + 1) * SCOLS],
                              in_=xs[0, m][:, 860:SCOLS]).then_inc(s_x0b2, 16)
            eng.dma_start(out=wt_t[:, 1536:1920], in_=wt[:, 1536:1920]).then_inc(s_wt[2], 16)
            eng.dma_start(out=wt_t[:, 1920:2304], in_=wt[:, 1920:2304]).then_inc(s_wt[3], 16)
            eng.dma_start(out=wt_t[:, 2304:2688], in_=wt[:, 2304:2688]).then_inc(s_wt[4], 16)
            eng.dma_start(out=wt_t[:, 2688:3072], in_=wt[:, 2688:3072]).then_inc(s_wt[5], 16)
            for m in range(4):
                eng.dma_start(out=xs_t[2][:, m * SCOLS:(m + 1) * SCOLS],
                              in_=xs[2, m]).then_inc(s_x[2], 16)

            # dummy activation: forces the Identity table load (~2.7us)
            # to happen during the initial DMA window
            if N_WU:
                eng.wait_ge(s_wu, 1)
                eng.activation(scr[:, :], wu[:, 0:16], Id)
            eng.wait_ge(s_cst, 16)

            for g in range(N_GRP):
                i, c, q, h = gidx(g)
                eng.wait_ge(s_mm, 4 * g + 2)
                if g >= 2:
                    eng.wait_ge(s_d2, 2 * g - 2)  # c2 tile reuse
                bias = 0.0 if "c2f" in KO else bv_t[:, c:c + 1]
                eng.activation(c2_view(h), bank_rd(h, 1), Id,
                               bias=bias).then_inc(s_c2, 1)
                eng.wait_ge(s_mm, 4 * g + 3)
                if g >= 2:
                    eng.wait_ge(s_d34, g - 1)  # b tile reuse
                eng.activation(il_view(b_t[h])[:, 0], bank_rd(h, 2),
                               Id).then_inc(s_b, 1)
                eng.wait_ge(s_mm, 4 * g + 4)
                eng.activation(il_view(b_t[h])[:, 1], bank_rd(h, 3),
                               Id).then_inc(s_b, 1)


        @block.vector
        def _(eng):
            eng.memset(wu[:, :], 0.0).then_inc(s_wu, 1)
            for g in range(N_GRP):
                i, c, q, h = gidx(g)
                # C2(g) done implies banks 1,2 of g are done
                eng.wait_ge(s_c2, g + 1)
                if g >= 2:
                    eng.wait_ge(s_d34, g - 1)  # t tile reuse (WAR)
                if "op12" in KO:
                    t0 = t_t[h][:, 0:GQ * W].rearrange("p (r c) -> p r c", c=W)
                    t1 = t_t[h][:, GQ * W:STG].rearrange("p (r c) -> p r c", c=W)
                else:
                    t0 = il_view(t_t[h])[:, 0]
                    t1 = il_view(t_t[h])[:, 1]
                eng.tensor_tensor(t0, bank_rd(h, 0), c2_view(h),
                                  op=mybir.AluOpType.add).then_inc(s_d2, 1)
                # odd-row partial from B3's fp16 copy of bank3 (-M3), not
                # the PSUM bank itself: each PSUM bank must have exactly
                # one reader engine (ACT+DVE same-bank access collides)
                eng.wait_ge(s_b, 2 * g + 1)
                eng.tensor_tensor(t1, il_view(b_t[h])[:, 0], c2_view(h),
                                  op=mybir.AluOpType.add).then_inc(s_d2, 1)
                eng.wait_ge(s_b, 2 * g + 2)  # B4 landed in b tile
                eng.wait_ge(s_d2, 2 * g + 2)  # own op1/op2 writeback drained
                b_ic = 2 * i + c
                if b_ic >= 4:
                    eng.wait_ge(s_out[b_ic % 4], 64 * (b_ic // 4))  # slot reuse
                o34 = (scr34[:, :] if "op34" in KO
                       else st_t[b_ic % 4][:, q * STG:(q + 1) * STG])
                eng.tensor_tensor(o34, t_t[h][:, :], b_t[h][:, :],
                                  op=mybir.AluOpType.subtract).then_inc(s_d34, 1)

        @block.sync
        def _(eng):
            # ring B (SP HWDGE): img0 streams 3-4, bias, img1, img3
            eng.dma_start(out=bv_t[:, :], in_=bv[:, :]).then_inc(s_cst, 16)
            for m in range(2, 4):
                eng.dma_start(out=xs_t[0][:, m * SCOLS:m * SCOLS + 456],
                              in_=xs[0, m][:, 0:456]).then_inc(s_x0p[m], 16)
            for m in range(2, 4):
                eng.dma_start(out=xs_t[0][:, m * SCOLS + 456:m * SCOLS + 860],
                              in_=xs[0, m][:, 456:860]).then_inc(s_x0b1, 16)
            for m in range(2, 4):
                eng.dma_start(out=xs_t[0][:, m * SCOLS + 860:(m + 1) * SCOLS],
                              in_=xs[0, m][:, 860:SCOLS]).then_inc(s_x0b2, 16)
            for i in (1, 3):
                for m in range(4):
                    eng.dma_start(out=xs_t[i][:, m * SCOLS:(m + 1) * SCOLS],
                                  in_=xs[i, m]).then_inc(s_x[i], 16)
            for g in range(N_GRP):
                i, c, q, h = gidx(g)
                eng.wait_ge(s_d34, g + 1)
                slot = (2 * i + c) % 4
                eng.dma_start(
                    out=y[i, c * 128:(c + 1) * 128, q * 14:(q + 1) * 14, :],
                    in_=st_t[slot][:, q * STG:(q + 1) * STG],
                ).then_inc(s_out[slot], 16)

        @block.gpsimd
        def _(eng):
            nf = [sum(1 for g in range(N_GRP)
                      if (2 * (g // 8) + (g // 4) % 2) % 4 == j)
                  for j in range(4)]
            for j in range(4):
                if nf[j]:
                    eng.wait_ge(s_out[j], 16 * nf[j])

        @block.tensor
        def _(eng):
            eng.wait_ge(s_wu, 1)
            for _ in range(N_WU):
                nc.tensor.matmul(wu_ps, wu[:, 0:128], wu[:, :],
                                 start=True, stop=True)
            for g in range(N_GRP):
                i, c, q, h = gidx(g)
                if g == 1:
                    eng.wait_ge(s_x0b1, 64)
                if g == 4:
                    eng.wait_ge(s_x0b2, 64)

                if g >= 8 and g % 8 == 0:
                    eng.wait_ge(s_x[i], 64)
                if g >= 2:
                    eng.wait_ge(s_d2, 2 * g - 2)
                    eng.wait_ge(s_b, 2 * g - 2)
                for m in range(4):
                    if g == 0:
                        if m == 0:
                            eng.wait_ge(s_wt[0], 16)
                        elif m == 1:
                            eng.wait_ge(s_wt[1], 16)
                        eng.wait_ge(s_x0p[m], 16)
                    if g == 2:
                        eng.wait_ge(s_wt[2 + m], 16)
                    blk = (c * 4 + m) * 3
                    off = m * SCOLS + q * SFREE
                    mi = None
                    for kw in range(3):
                        mi = nc.tensor.matmul(
                            bank_mm(h, m),
                            wt_t[:, (blk + kw) * 128:(blk + kw + 1) * 128],
                            xs_t[i][:, off + kw:off + kw + SFREE],
                            start=(kw == 0),
                            stop=(kw == 2),
                        )
                    mi.then_inc(s_mm, 1)

        nc.all_engine_barrier()
        nc.gpsimd.dma_reset()
        nc.gpsimd.sem_clear(nc._kernel_sem_range)

    nc.compile()
    return nc


def prep_inputs(x, w_q, s, bias, passes=None):
    """Full inputs -> list of 8 per-core in_maps (numpy)."""
    x = np.asarray(x, dtype=np.float32)
    wq = np.asarray(w_q).astype(np.float32)
    s = np.asarray(s, dtype=np.float32).reshape(C_OUT)
    bias = np.asarray(bias, dtype=np.float32).reshape(C_OUT)

    # Winograd F(2,3) row streams from the zero-padded image P[58, 57]
    # (P[p, q] = x[p-1, q-1]; row 0 / col 0 are the top/left pad, the
    # bottom pad is row 57, the right pad is col 0 of the next row).
    P = np.zeros((N_IMG, C_IN, H + 2, SROW), np.float32)
    P[:, :, 1:H + 1, 1:W + 1] = x.reshape(N_IMG, C_IN, H, W)
    S = np.stack([
        P[:, :, 0:56:2] - P[:, :, 2:58:2],  # S1
        P[:, :, 1:57:2] + P[:, :, 2:58:2],  # S2
        P[:, :, 2:58:2] - P[:, :, 1:57:2],  # S3
        P[:, :, 1:57:2] - P[:, :, 3:58:2],  # S4 (row 57 is the bottom pad)
    ], axis=1)  # [N, 4, C_IN, 28, 57]
    xs = np.zeros((N_IMG, 4, C_IN, SCOLS), np.float16)
    xs[:, :, :, :NPAIR * SROW] = S.reshape(N_IMG, 4, C_IN, NPAIR * SROW)
    xs = xs.reshape(N_CORES, IMG, 4, C_IN, SCOLS)

    # transformed weights, scale folded, V3 negated (bank3 = -M3),
    # blocks ordered (chunk, m, kw), each [C_IN, 128] pre-transposed
    w = wq * s[:, None, None, None]  # [256, 128, 3, 3]
    g0, g1, g2 = w[:, :, 0, :], w[:, :, 1, :], w[:, :, 2, :]
    V = np.stack([
        g0,
        (g0 + g1 + g2) * 0.5,
        -(g0 - g1 + g2) * 0.5,
        g2,
    ], axis=0)  # [4, 256, C_IN, 3]
    V = V.reshape(4, N_CH, 128, C_IN, 3)
    wt = np.ascontiguousarray(np.transpose(V, (3, 1, 0, 4, 2))).reshape(
        C_IN, 24 * 128).astype(np.float16)

    bv = np.ascontiguousarray(bias.reshape(N_CH, 128).T)

    in_maps = []
    for core in range(N_CORES):
        in_maps.append({"xs": np.ascontiguousarray(xs[core]),
                        "wt": wt, "bv": bv})
    return in_maps


_NC_CACHE: dict[str, bacc.Bacc] = {}


def get_nc(*_args, **_kwargs) -> bacc.Bacc:
    if "nc" not in _NC_CACHE:
        _NC_CACHE["nc"] = build_nc()
    return _NC_CACHE["nc"]


def run(inputs, trace: bool = False, **run_kwargs):
    """Returns (full_output, BassKernelResults)."""
    from concourse.bass_utils import run_bass_kernel_spmd

    run_kwargs.pop("passes", None)
    nc = get_nc()
    in_maps = prep_inputs(**inputs)
    res = run_bass_kernel_spmd(nc, in_maps, list(range(N_CORES)),
                               trace=trace, **run_kwargs)
    out = np.concatenate([np.asarray(res.results[i]["y"])
                          for i in range(N_CORES)], axis=0)
    return out, res


def kernel(**inputs) -> np.ndarray:
    out, _ = run(inputs)
    return out


# revision 42
# speedup vs baseline: 1.1473x; 1.1473x over previous
"""BitConv2d inference kernel for Trainium2 (8 NeuronCores, SPMD).

Problem: y = conv2d(x, w_q.float(), stride=1, pad=1) * s + bias
  x:    (32, 128, 56, 56) f32
  w_q:  (256, 128, 3, 3) ternary {-1,0,+1} (int8 or int32)
  s:    (256, 1, 1) f32
  bias: (256,) f32
  y:    (32, 256, 56, 56) f32

Strategy: data-parallel over batch (4 images per core) + Winograd F(2,3)
along H. The 3 kh-taps collapse into 4 host-precomputed fp16 "streams"
per image (S1..S4 = +/- combinations of adjacent padded rows), so each
pair of output rows needs only 4 matmul products instead of 6:

  bank1 = M1 = sum_kw V1[kw]^T S1(shift kw)      (3 matmuls)
  bank2 = M2                                      (3)
  bank3 = -M3  (V3 pre-negated on host)           (3)
  bank4 = M4                                      (3)
  even rows E = M1 + M2 + M3 + b,  odd rows O = M2 - M3 - M4 + b

12 matmuls per 16 output rows vs 18 direct = 1.5x fewer PE cycles.
The kw taps reuse the baseline's stride-57 shifted-slice trick (stream
rows are 57 wide; the zero column between rows is the shared pad).

Combine per group of 7 row-pairs (free dim 399 = 7x57, one PSUM bank):
  ACT: C2 = Id(bank2 + bias) -> fp16   (bias rides into both parities)
       B  = Id([bank3; bank4])  -> fp16, rows interleaved
  DVE: T(even) = bank1 + C2, T(odd) = bank3 + C2   (rows interleaved)
       stage   = T - B        (dense fp16 2x op; E = T-(-M3), O = T-M4)
All junk pad columns are dropped at the PSUM->SBUF boundary, so the
staging tile and the output DMA are fully contiguous (14 rows x 224B
per partition per group), line-rate both sides.

Scales s are folded into the transformed weights on the host.
Per-core per-group engine budget @2.4GHz: PE 1995ns, DVE ~1700ns,
ACT ~1570ns -> PE-bound, ~32 groups ~= 67-70us (vs 114.5us baseline).
"""

import os
from contextlib import ExitStack

import numpy as np

import concourse.bass as bass
import concourse.mybir as mybir
from concourse import bacc

# Problem constants (hardcoded per contract)
N_IMG, C_IN, C_OUT, H, W = 32, 128, 256, 56, 56
N_CORES = 8
IMG = N_IMG // N_CORES  # 4 images per core
IMG_PER_CORE = IMG  # test.py compat
PASSES = 1  # test.py compat (no residual pass needed: rel err ~1e-3)
N_CH = C_OUT // 128  # 2 cout chunks

SROW = 57  # stream row stride (56 cols + shared pad col)
NPAIR = 28  # output row pairs per image
GQ = 7  # row pairs per group
NGRP = NPAIR // GQ  # 4 groups per (img, chunk)
SFREE = GQ * SROW  # 399, matmul free dim (<= 512 fp32 PSUM bank)
SCOLS = NPAIR * SROW + 8  # 1604: stream cols + tap-overshoot slack
STG = 2 * GQ * W  # 784: staging elems per group (14 rows x 56)
IMG_RUN = int(os.environ.get("WINO_IMGS", "4"))  # debug: images to process
N_GRP = int(os.environ.get("WINO_GRPS", str(IMG_RUN * N_CH * NGRP)))
N_WU = int(os.environ.get("WINO_WARMUP", "30"))
KO = set(os.environ.get("WINO_KO", "").split(","))  # debug knockouts
N_WARMUP = 12
WU_FREE = 256

f16 = mybir.dt.float16
f32 = mybir.dt.float32


def build_nc() -> bacc.Bacc:
    nc = bacc.Bacc("TRN2", target_bir_lowering=False, debug=False)

    xs = nc.dram_tensor("xs", [IMG, 4, C_IN, SCOLS], f16,
                        kind="ExternalInput").ap()
    wt = nc.dram_tensor("wt", [C_IN, 24 * 128], f16, kind="ExternalInput").ap()
    bv = nc.dram_tensor("bv", [128, N_CH], f32, kind="ExternalInput").ap()
    y = nc.dram_tensor("y", [IMG, C_OUT, H, W], f32, kind="ExternalOutput").ap()

    # static SBUF
    xs_t = [nc.alloc_sbuf_tensor(f"sxs{i}", [C_IN, 4 * SCOLS], f16).ap()
            for i in range(IMG)]
    wt_t = nc.alloc_sbuf_tensor("swt", [C_IN, 24 * 128], f16).ap()
    bv_t = nc.alloc_sbuf_tensor("sbv", [128, N_CH], f32).ap()
    wu = nc.alloc_sbuf_tensor("swu", [128, WU_FREE], f16).ap()
    scr = nc.alloc_sbuf_tensor("sscr", [128, 16], f16).ap()
    scr34 = nc.alloc_sbuf_tensor("sscr34", [128, STG], f32).ap()
    c2_t = [nc.alloc_sbuf_tensor(f"sc2{h}", [128, GQ * W], f16).ap()
            for h in range(2)]
    t_t = [nc.alloc_sbuf_tensor(f"st{h}", [128, STG], f16).ap()
           for h in range(2)]
    b_t = [nc.alloc_sbuf_tensor(f"sb{h}", [128, STG], f16).ap()
           for h in range(2)]
    st_t = [nc.alloc_sbuf_tensor(f"sst{j}", [128, NGRP * STG], f32).ap()
            for j in range(4)]
    ps = nc.alloc_psum_tensor("ps", [128, 4096], f32).ap()

    def bank_mm(h, m):  # matmul target: one PSUM bank, dense
        base = (4 * h + m) * 512
        return ps[:, base:base + SFREE]

    def bank_rd(h, m):  # junk-column-free read view [p, 7, 56]
        base = (4 * h + m) * 512
        return ps[:, base:base + SFREE].rearrange(
            "p (r c) -> p r c", c=SROW)[:, :, 0:W]

    def b34_in(h):  # banks 3,4 as [p, 2, 7, 56]
        base = (4 * h + 2) * 512
        return ps[:, base:base + 1024].rearrange(
            "p (b x) -> p b x", b=2)[:, :, 0:SFREE].rearrange(
            "p b (r c) -> p b r c", c=SROW)[:, :, :, 0:W]

    def il_view(tile):  # [128, 784] -> row-interleaved [p, parity, 7, 56]
        return tile.rearrange("p (r t c) -> p t r c", t=2, c=W)

    def c2_view(h):
        return c2_t[h].rearrange("p (r c) -> p r c", c=W)

    wu_ps = ps[:, 7 * 512:7 * 512 + WU_FREE]

    Id = mybir.ActivationFunctionType.Identity

    def gidx(g):
        return g // 8, (g // 4) % 2, g % 4, g % 2  # img, chunk, grp, half

    with ExitStack() as ctx:
        s_wu = ctx.enter_context(nc.semaphore("s_wu"))
        s_wt = [ctx.enter_context(nc.semaphore(f"s_wt{k}")) for k in range(6)]
        s_x0p = [ctx.enter_context(nc.semaphore(f"s_x0p{m}")) for m in range(4)]
        s_x0b1 = ctx.enter_context(nc.semaphore("s_x0b1"))
        s_x0b2 = ctx.enter_context(nc.semaphore("s_x0b2"))
        s_x = [None] + [ctx.enter_context(nc.semaphore(f"s_x{i}"))
                        for i in range(1, IMG)]
        s_cst = ctx.enter_context(nc.semaphore("s_cst"))
        s_mm = ctx.enter_context(nc.semaphore("s_mm"))
        s_c2 = ctx.enter_context(nc.semaphore("s_c2"))
        s_b = ctx.enter_context(nc.semaphore("s_b"))
        s_d2 = ctx.enter_context(nc.semaphore("s_d2"))
        s_d34 = ctx.enter_context(nc.semaphore("s_d34"))
        s_out = [ctx.enter_context(nc.semaphore(f"s_out{j}"))
                 for j in range(4)]
        block = ctx.enter_context(nc.Block())

        @block.scalar
        def _(eng):
            # input DMAs on the ACT HWDGE ring; critical prefix first.
            # weight blocks ordered (chunk, m, kw); group 0 consumes
            # m-banks in order, so stage the DMAs to match.
            # ring A (ACT HWDGE): img0 streams 1-2, all weights, img2
            eng.dma_start(out=wt_t[:, 0:384], in_=wt[:, 0:384]).then_inc(s_wt[0], 16)
            for m in range(2):
                eng.dma_start(out=xs_t[0][:, m * SCOLS:m * SCOLS + 456],
                              in_=xs[0, m][:, 0:456]).then_inc(s_x0p[m], 16)
            eng.dma_start(out=wt_t[:, 384:1536], in_=wt[:, 384:1536]).then_inc(s_wt[1], 16)
            for m in range(2):
                eng.dma_start(out=xs_t[0][:, m * SCOLS + 456:m * SCOLS + 860],
                              in_=xs[0, m][:, 456:860]).then_inc(s_x0b1, 16)
            for m in range(2):
                eng.dma_start(out=xs_t[0][:, m * SCOLS + 860:(m + 1) * SCOLS],
                              in_=xs[0, m][:, 860:SCOLS]).then_inc(s_x0b2, 16)
            eng.dma_start(out=wt_t[:, 1536:1920], in_=wt[:, 1536:1920]).then_inc(s_wt[2], 16)
            eng.dma_start(out=wt_t[:, 1920:2304], in_=wt[:, 1920:2304]).then_inc(s_wt[3], 16)
            eng.dma_start(out=wt_t[:, 2304:2688], in_=wt[:, 2304:2688]).then_inc(s_wt[4], 16)
            eng.dma_start(out=wt_t[:, 2688:3072], in_=wt[:, 2688:3072]).then_inc(s_wt[5], 16)
            for m in range(4):
                eng.dma_start(out=xs_t[2][:, m * SCOLS:(m + 1) * SCOLS],
                              in_=xs[2, m]).then_inc(s_x[2], 16)

            # dummy activation: forces the Identity table load (~2.7us)
            # to happen during the initial DMA window
            if N_WU:
                eng.wait_ge(s_wu, 1)
                eng.activation(scr[:, :], wu[:, 0:16], Id)
            eng.wait_ge(s_cst, 16)

            for g in range(N_GRP):
                i, c, q, h = gidx(g)
                eng.wait_ge(s_mm, 4 * g + 2)
                if g >= 2:
                    eng.wait_ge(s_d2, 2 * g - 2)  # c2 tile reuse
                bias = 0.0 if "c2f" in KO else bv_t[:, c:c + 1]
                eng.activation(c2_view(h), bank_rd(h, 1), Id,
                               bias=bias).then_inc(s_c2, 1)
                eng.wait_ge(s_mm, 4 * g + 3)
                if g >= 2:
                    eng.wait_ge(s_d34, g - 1)  # b tile reuse
                eng.activation(il_view(b_t[h])[:, 0], bank_rd(h, 2),
                               Id).then_inc(s_b, 1)
                eng.wait_ge(s_mm, 4 * g + 4)
                eng.activation(il_view(b_t[h])[:, 1], bank_rd(h, 3),
                               Id).then_inc(s_b, 1)


        @block.vector
        def _(eng):
            eng.memset(wu[:, :], 0.0).then_inc(s_wu, 1)
            for g in range(N_GRP):
                i, c, q, h = gidx(g)
                # C2(g) done implies banks 1,2 of g are done
                eng.wait_ge(s_c2, g + 1)
                if g >= 2:
                    eng.wait_ge(s_d34, g - 1)  # t tile reuse (WAR)
                if "op12" in KO:
                    t0 = t_t[h][:, 0:GQ * W].rearrange("p (r c) -> p r c", c=W)
                    t1 = t_t[h][:, GQ * W:STG].rearrange("p (r c) -> p r c", c=W)
                else:
                    t0 = il_view(t_t[h])[:, 0]
                    t1 = il_view(t_t[h])[:, 1]
                eng.tensor_tensor(t0, bank_rd(h, 0), c2_view(h),
                                  op=mybir.AluOpType.add).then_inc(s_d2, 1)
                # odd-row partial from B3's fp16 copy of bank3 (-M3), not
                # the PSUM bank itself: each PSUM bank must have exactly
                # one reader engine (ACT+DVE same-bank access collides)
                eng.wait_ge(s_b, 2 * g + 1)
                eng.tensor_tensor(t1, il_view(b_t[h])[:, 0], c2_view(h),
                                  op=mybir.AluOpType.add).then_inc(s_d2, 1)
                eng.wait_ge(s_b, 2 * g + 2)  # B4 landed in b tile
                eng.wait_ge(s_d2, 2 * g + 2)  # own op1/op2 writeback drained
                b_ic = 2 * i + c
                if b_ic >= 4:
                    eng.wait_ge(s_out[b_ic % 4], 64 * (b_ic // 4))  # slot reuse
                o34 = (scr34[:, :] if "op34" in KO
                       else st_t[b_ic % 4][:, q * STG:(q + 1) * STG])
                eng.tensor_tensor(o34, t_t[h][:, :], b_t[h][:, :],
                                  op=mybir.AluOpType.subtract).then_inc(s_d34, 1)

        @block.sync
        def _(eng):
            # ring B (SP HWDGE): img0 streams 3-4, bias, img1, img3
            eng.dma_start(out=bv_t[:, :], in_=bv[:, :]).then_inc(s_cst, 16)
            for m in range(2, 4):
                eng.dma_start(out=xs_t[0][:, m * SCOLS:m * SCOLS + 456],
                              in_=xs[0, m][:, 0:456]).then_inc(s_x0p[m], 16)
            for m in range(2, 4):
                eng.dma_start(out=xs_t[0][:, m * SCOLS + 456:m * SCOLS + 860],
                              in_=xs[0, m][:, 456:860]).then_inc(s_x0b1, 16)
            for m in range(2, 4):
                eng.dma_start(out=xs_t[0][:, m * SCOLS + 860:(m + 1) * SCOLS],
                              in_=xs[0, m][:, 860:SCOLS]).then_inc(s_x0b2, 16)
            for i in (1, 3):
                for m in range(4):
                    eng.dma_start(out=xs_t[i][:, m * SCOLS:(m + 1) * SCOLS],
                                  in_=xs[i, m]).then_inc(s_x[i], 16)
            for g in range(N_GRP):
                i, c, q, h = gidx(g)
                eng.wait_ge(s_d34, g + 1)
                slot = (2 * i + c) % 4
                eng.dma_start(
                    out=y[i, c * 128:(c + 1) * 128, q * 14:(q + 1) * 14, :],
                    in_=st_t[slot][:, q * STG:(q + 1) * STG],
                ).then_inc(s_out[slot], 16)

        @block.gpsimd
        def _(eng):
            nf = [sum(1 for g in range(N_GRP)
                      if (2 * (g // 8) + (g // 4) % 2) % 4 == j)
                  for j in range(4)]
            for j in range(4):
                if nf[j]:
                    eng.wait_ge(s_out[j], 16 * nf[j])

        @block.tensor
        def _(eng):
            eng.wait_ge(s_wu, 1)
            for _ in range(N_WU):
                nc.tensor.matmul(wu_ps, wu[:, 0:128], wu[:, :],
                                 start=True, stop=True)
            for g in range(N_GRP):
                i, c, q, h = gidx(g)
                if g == 1:
                    eng.wait_ge(s_x0b1, 64)
                if g == 2:
                    eng.wait_ge(s_x0b2, 64)

                if g >= 8 and g % 8 == 0:
                    eng.wait_ge(s_x[i], 64)
                if g >= 2:
                    eng.wait_ge(s_d2, 2 * g - 2)
                    eng.wait_ge(s_b, 2 * g - 2)
                for m in range(4):
                    if g == 0:
                        if m == 0:
                            eng.wait_ge(s_wt[0], 16)
                        elif m == 1:
                            eng.wait_ge(s_wt[1], 16)
                        eng.wait_ge(s_x0p[m], 16)
                    if g == 4 and m < 4:
                        eng.wait_ge(s_wt[2 + m], 16)
                    blk = (c * 4 + m) * 3
                    off = m * SCOLS + q * SFREE
                    mi = None
                    for kw in range(3):
                        mi = nc.tensor.matmul(
                            bank_mm(h, m),
                            wt_t[:, (blk + kw) * 128:(blk + kw + 1) * 128],
                            xs_t[i][:, off + kw:off + kw + SFREE],
                            start=(kw == 0),
                            stop=(kw == 2),
                        )
                    mi.then_inc(s_mm, 1)

        nc.all_engine_barrier()
        nc.gpsimd.dma_reset()
        nc.gpsimd.sem_clear(nc._kernel_sem_range)

    nc.compile()
    return nc


def prep_inputs(x, w_q, s, bias, passes=None):
    """Full inputs -> list of 8 per-core in_maps (numpy)."""
    x = np.asarray(x, dtype=np.float32)
    wq = np.asarray(w_q).astype(np.float32)
    s = np.asarray(s, dtype=np.float32).reshape(C_OUT)
    bias = np.asarray(bias, dtype=np.float32).reshape(C_OUT)

    # Winograd F(2,3) row streams from the zero-padded image P[58, 57]
    # (P[p, q] = x[p-1, q-1]; row 0 / col 0 are the top/left pad, the
    # bottom pad is row 57, the right pad is col 0 of the next row).
    P = np.zeros((N_IMG, C_IN, H + 2, SROW), np.float32)
    P[:, :, 1:H + 1, 1:W + 1] = x.reshape(N_IMG, C_IN, H, W)
    S = np.stack([
        P[:, :, 0:56:2] - P[:, :, 2:58:2],  # S1
        P[:, :, 1:57:2] + P[:, :, 2:58:2],  # S2
        P[:, :, 2:58:2] - P[:, :, 1:57:2],  # S3
        P[:, :, 1:57:2] - P[:, :, 3:58:2],  # S4 (row 57 is the bottom pad)
    ], axis=1)  # [N, 4, C_IN, 28, 57]
    xs = np.zeros((N_IMG, 4, C_IN, SCOLS), np.float16)
    xs[:, :, :, :NPAIR * SROW] = S.reshape(N_IMG, 4, C_IN, NPAIR * SROW)
    xs = xs.reshape(N_CORES, IMG, 4, C_IN, SCOLS)

    # transformed weights, scale folded, V3 negated (bank3 = -M3),
    # blocks ordered (chunk, m, kw), each [C_IN, 128] pre-transposed
    w = wq * s[:, None, None, None]  # [256, 128, 3, 3]
    g0, g1, g2 = w[:, :, 0, :], w[:, :, 1, :], w[:, :, 2, :]
    V = np.stack([
        g0,
        (g0 + g1 + g2) * 0.5,
        -(g0 - g1 + g2) * 0.5,
        g2,
    ], axis=0)  # [4, 256, C_IN, 3]
    V = V.reshape(4, N_CH, 128, C_IN, 3)
    wt = np.ascontiguousarray(np.transpose(V, (3, 1, 0, 4, 2))).reshape(
        C_IN, 24 * 128).astype(np.float16)

    bv = np.ascontiguousarray(bias.reshape(N_CH, 128).T)

    in_maps = []
    for core in range(N_CORES):
        in_maps.append({"xs": np.ascontiguousarray(xs[core]),
                        "wt": wt, "bv": bv})
    return in_maps


_NC_CACHE: dict[str, bacc.Bacc] = {}


def get_nc(*_args, **_kwargs) -> bacc.Bacc:
    if "nc" not in _NC_CACHE:
        _NC_CACHE["nc"] = build_nc()
    return _NC_CACHE["nc"]


def run(inputs, trace: bool = False, **run_kwargs):
    """Returns (full_output, BassKernelResults)."""
    from concourse.bass_utils import run_bass_kernel_spmd

    run_kwargs.pop("passes", None)
    nc = get_nc()
    in_maps = prep_inputs(**inputs)
    res = run_bass_kernel_spmd(nc, in_maps, list(range(N_CORES)),
                               trace=trace, **run_kwargs)
    out = np.concatenate([np.asarray(res.results[i]["y"])
                          for i in range(N_CORES)], axis=0)
    return out, res


def kernel(**inputs) -> np.ndarray:
    out, _ = run(inputs)
    return out
